# revision 24
# baseline (speedup 1.0000x reference)
"""Decoder block (single-head causal attention + GELU FFN) on 8 TRN2 NeuronCores.

Sharding: data parallel over batch (2 cores per batch), with the K AND V
projections token-split across the pair (each core projects its own half of
the sequence, then a pairwise AllGather shares both). Core c handles batch
b = c//2 and 1024 query tokens of that batch, chosen as four 256-token chunks
that balance the causal-attention workload:
  even cores (half 0): chunks 0, 3, 4, 7
  odd  cores (half 1): chunks 1, 2, 5, 6
The slot pairing makes the static k-tile counts per slot (4, 8, 12, 16) cover
both cores' needs with minimal waste (ideal is 36 tiles vs 40; the gap is
zeroed by the data-driven qpos mask). The SPMD program is identical on every
core; all per-core differences are data.

Performance structure (v2):
  - every matmul operand is fp16 (same PE rate as fp32r, half the DMA/SBUF)
  - V projection runs over the core's OWN half only (was: full sequence);
    K and V are exchanged by two pairwise AllGathers (K fires early, V later;
    P2 is restructured into scores-first/PV-second passes so the V gather
    latency hides behind all-slot score computation)
  - all weights / x tiles are multi-dim SBUF tiles filled by ONE or TWO big
    DMAs each (descriptor-generation on the issuing engine was costing
    ~630ns per 128KB tile; big transfers cut the issue count ~7x)
  - host pre-arranges every DRAM operand so each big DMA is contiguous per
    partition line (16KB runs)
  - Q/Wo/FFN matmuls interleave the two 512-token column blocks under one
    stationary weight load (halves LDWEIGHTS pressure; the pair partner's
    load hides under the 213ns FD=512 matmul)
  - P4 stages GELU results per 512-row block and stores once per block from
    the sync engine (64 -> 8 stores)
  - scalar engine runs only Identity in P1 and only Exp in P2 (activation
    table reloads cost 1.3us each)
"""

import numpy as np
import ml_dtypes

_E4M3 = ml_dtypes.float8_e4m3fn

D = 1024  # model dim
S = 2048  # sequence length
B = 4  # batch
M = 4096  # FFN dim
CH = 256  # q chunk (slot) size
NSLOT = 4  # q slots per core
NDT = D // 128  # 8 d-tiles
N_CORES = 8
NKT = [4, 8, 12, 16]  # k-tiles per slot (static max over the two paired cores)

_PROGRAM = None  # cached compiled program


def _build_program():
    import sys

    if "/opt/trn_rl_repo" not in sys.path:
        sys.path.insert(0, "/opt/trn_rl_repo")
    import concourse.bass as bass
    import concourse.tile as tile
    import concourse.mybir as mybir
    from concourse import bacc
    from concourse.bass import ts

    dt = mybir.dt
    AF = mybir.ActivationFunctionType
    ALU = mybir.AluOpType
    F32, F32R, F16, F8 = dt.float32, dt.float32r, dt.float16, dt.float8e4
    DR = mybir.MatmulPerfMode.DoubleRow

    nc = bacc.Bacc("TRN2", target_bir_lowering=False, debug=False, num_devices=8)

    # ---------------- DRAM I/O (all host-pre-arranged layouts) ----------------
    # fp16 weights: [128, i(8), 1024] with [p, i, c] = W.T[i*128+p, c]
    wvT = nc.dram_tensor("wvT", [128, NDT, D], F16, kind="ExternalInput").ap()
    woT = nc.dram_tensor("woT", [128, NDT, D], F16, kind="ExternalInput").ap()
    # fp8 QK weights in DoubleRow pair layout, host-scaled by 64:
    # [p, i2, s, c] = e4m3(64 * W.T[(2*i2+s)*128+p, c])
    wq8T = nc.dram_tensor("wq8T", [128, 4, 2, D], F8, kind="ExternalInput").ap()
    wk8T = nc.dram_tensor("wk8T", [128, 4, 2, D], F8, kind="ExternalInput").ap()
    # FFN weight: [mb(8), 128, i(8), 512] with [mb, p, i, c] = Wf.T[i*128+p, mb*512+c]
    wfT = nc.dram_tensor("wfT", [M // 512, 128, NDT, 512], F16, kind="ExternalInput").ap()
    # x, own-half tokens in k order: [p, i, t] = x.T[i*128+p, half*1024+t]
    xaT = nc.dram_tensor("xaT", [128, NDT, S // 2], F16, kind="ExternalInput").ap()
    # fp8 x copies in DoubleRow pair layout (for the Q/K projections)
    xa8T = nc.dram_tensor("xa8T", [128, 4, 2, S // 2], F8, kind="ExternalInput").ap()
    xq8T = nc.dram_tensor("xq8T", [128, 4, 2, 4 * CH], F8, kind="ExternalInput").ap()
    bq = nc.dram_tensor("bq", [128, D // 128], F32, kind="ExternalInput").ap()
    bk = nc.dram_tensor("bk", [128, D // 128], F32, kind="ExternalInput").ap()
    bo2 = nc.dram_tensor("bo2", [128, D // 128], F32, kind="ExternalInput").ap()
    bfT = nc.dram_tensor("bfT", [128, M // 128], F32, kind="ExternalInput").ap()
    qpos = nc.dram_tensor("qpos", [1, 4 * CH], F32R, kind="ExternalInput").ap()
    bf_row = nc.dram_tensor("bf_row", [1, M], F32R, kind="ExternalInput").ap()
    iota_kt = nc.dram_tensor("iota_kt", [128, S // 128], F32, kind="ExternalInput").ap()
    # output: [mb(8), 128p, t8(8), 512c] = ff[(t8//4)*512+(t8%4)*128+p, mb*512+c]
    ffT = nc.dram_tensor("ffT", [M // 512, 128, 8, 512], F16, kind="ExternalOutput").ap()

    with tile.TileContext(nc) as tc:
        with (
            tc.tile_pool(name="const", bufs=1) as cpool,
            tc.tile_pool(name="psum", bufs=1, space="PSUM") as pspool,
        ):
            # ---------------- constants (scalar engine issues these) --------
            ones_col_bf = cpool.tile([128, 1], F16, name="ones_col_bf", tag="ones_col_bf")
            nc.vector.memset(ones_col_bf[:], 1.0)
            ones_row_f = cpool.tile([1, 128], F32, name="ones_row_f", tag="ones_row_f")
            nc.vector.memset(ones_row_f[:], 1.0)
            ones_row = cpool.tile([1, 128], F32R, name="ones_row", tag="ones_row")
            nc.vector.tensor_copy(ones_row[:], ones_row_f[:])
            iota_sb = cpool.tile([128, S // 128], F32, name="iota", tag="iota")
            nc.scalar.dma_start(iota_sb[:], iota_kt[:])
            bq_sb = cpool.tile([128, D // 128], F32, name="bq", tag="bq")
            nc.scalar.dma_start(bq_sb[:], bq[:])
            bk_sb = cpool.tile([128, D // 128], F32, name="bk", tag="bk")
            nc.scalar.dma_start(bk_sb[:], bk[:])
            bo2_sb = cpool.tile([128, D // 128], F32, name="bo2", tag="bo2")
            nc.scalar.dma_start(bo2_sb[:], bo2[:])
            bf_sb = cpool.tile([128, M // 128], F32, name="bf", tag="bf")
            nc.scalar.dma_start(bf_sb[:], bfT[:])
            qpos_row = cpool.tile([1, 4 * CH], F32R, name="qpos_row", tag="qpos_row")
            nc.scalar.dma_start(qpos_row[:], qpos[:])
            bfr_sb = cpool.tile([1, M], F32R, name="bfr_sb", tag="bfr_sb")
            nc.scalar.dma_start(bfr_sb[:], bf_row[:])
            qposB = cpool.tile([128, 4 * CH], F32, name="qposB", tag="qposB")
            # V-pass inputs stream from scalar's queue at t0: they follow the
            # (tiny) const DMAs, so the critical wk8/xa8 pieces on gpsimd/sync
            # still see most of the wire, and wv/xa are in well before V needs
            # them (~25us)

            # ------------- long-lived tiles: one pool spanning P1..P4 -------
            with (
                tc.tile_pool(name="main", bufs=1) as mp,
                tc.tile_pool(name="dram", bufs=1, space="DRAM") as dram,
            ):
                # fp8 K^T in DoubleRow pair layout: [p, i2, s, tok],
                # contraction d = (2*i2+s)*128 + p
                kT8 = mp.tile([128, 4, 2, S], F8, name="kT8", tag="kT8")
                vt = mp.tile([128, 16, D], F16, name="vt", tag="vt")
                wo_sb = mp.tile([128, NDT, D], F16, name="wo", tag="wo")
                qT8 = [
                    mp.tile([128, 4, 2, 512], F8, name=f"qT8_{qb}", tag=f"qT8_{qb}")
                    for qb in range(2)
                ]
                attnT = [
                    [mp.tile([128, 512], F16, name=f"at{dt_}_{qb}", tag=f"at{dt_}_{qb}") for qb in range(2)]
                    for dt_ in range(NDT)
                ]
                # DRAM bounce buffers for the pairwise K and V AllGathers.
                # Each projection is gathered in two 1MB halves so the
                # collectives fire earlier and finish well before P2 needs
                # the peer's tokens.
                ka_in = dram.tile([128, 4, 2, 512], F8, name="ka_in", tag="ka_in")
                ka_out = dram.tile([2, 128, 4, 2, 512], F8, name="ka_out", tag="ka_out")
                kb_in = dram.tile([128, 4, 2, 512], F8, name="kb_in", tag="kb_in")
                kb_out = dram.tile([2, 128, 4, 2, 512], F8, name="kb_out", tag="kb_out")
                v_in = dram.tile([128, NDT, D], F16, name="v_in", tag="v_in")
                v_out = dram.tile([2, 128, NDT, D], F16, name="v_out", tag="v_out")

                def pair_gather(in_t, out_t):
                    nc.gpsimd.collective_compute(
                        "AllGather",
                        mybir.AluOpType.bypass,
                        replica_groups=[[0, 1], [2, 3], [4, 5], [6, 7]],
                        ins=[in_t[:].opt()],
                        outs=[out_t[:].opt()],
                    )

                # ---------------- P1 ----------------
                with tc.tile_pool(name="p1a", bufs=1) as p1a:
                    wk8_sb = p1a.tile([128, 4, 2, D], F8, name="wk8", tag="wk8")
                    wq8_sb = p1a.tile([128, 4, 2, D], F8, name="wq8", tag="wq8")
                    wv_sb = p1a.tile([128, NDT, D], F16, name="wv", tag="wv")
                    xa = p1a.tile([128, NDT, S // 2], F16, name="xa", tag="xa")
                    xa8 = p1a.tile([128, 4, 2, S // 2], F8, name="xa8", tag="xa8")
                    xq8 = p1a.tile([128, 4, 2, 4 * CH], F8, name="xq8", tag="xq8")
                    for h in range(2):
                        nc.scalar.dma_start(
                            wv_sb[:, ts(h, 4), :], wvT[:, ts(h, 4), :]
                        )
                        nc.scalar.dma_start(
                            xa[:, ts(h, 4), :], xaT[:, ts(h, 4), :]
                        )

                    # DMA issue is tiered: all in-flight DMAs share wire
                    # bandwidth, so only the critical-path wk/xa stream starts
                    # immediately (per-i granularity: compute starts on the
                    # first 256KB). Later streams are gated on compute
                    # sentinels and issued from the otherwise-idle vector
                    # engine so they cannot steal bandwidth early.
                    for h in range(2):
                        nc.gpsimd.dma_start(
                            wk8_sb[:, :, :, ts(h, 512)], wk8T[:, :, :, ts(h, 512)]
                        )
                        nc.sync.dma_start(
                            xa8[:, :, :, ts(h, 512)], xa8T[:, :, :, ts(h, 512)]
                        )


                    # ---- K projection over own half (kT staged at [:, :, 0:1024])
                    for tb in range(2):
                        for og in range(2):
                            ps4 = [
                                pspool.tile([128, 512], F32, name=f"at{j}", tag=f"at{j}", bufs=1)
                                for j in range(4)
                            ]
                            for i2 in range(4):
                                for j in range(4):
                                    nc.tensor.matmul(
                                        ps4[j][:],
                                        wk8_sb[:, i2, :, ts(og * 4 + j, 128)],
                                        xa8[:, i2, :, ts(tb, 512)],
                                        start=(i2 == 0), stop=(i2 == 3),
                                        perf_mode=DR,
                                    )
                            for j in range(4):
                                ot = og * 4 + j
                                nc.scalar.activation(
                                    kT8[:, ot // 2, ot % 2, ts(tb, 512)], ps4[j][:],
                                    AF.Identity, bias=bk_sb[:, ot : ot + 1],
                                    scale=1.0 / 64.0,
                                )
                            if tb == 0 and og == 0:
                                pass
                        # stage this token block and fire its K gather
                        kin = ka_in if tb == 0 else kb_in
                        nc.sync.dma_start(kin[:], kT8[:, :, :, ts(tb, 512)])
                        pair_gather(kin, ka_out if tb == 0 else kb_out)

                    # ---- V projection over own half (token-major, 2 banks/tt)
                    for tb in range(2):
                        for tt in range(4):
                            ps2 = [
                                pspool.tile([128, 512], F32, name=f"mm{ob}", tag="mm", bufs=4)
                                for ob in range(2)
                            ]
                            for i in range(NDT):
                                for ob in range(2):
                                    nc.tensor.matmul(
                                        ps2[ob][:],
                                        xa[:, i, tb * 512 + tt * 128 : tb * 512 + (tt + 1) * 128],
                                        wv_sb[:, i, ts(ob, 512)],
                                        start=(i == 0), stop=(i == NDT - 1),
                                    )
                            for ob in range(2):
                                nc.scalar.activation(
                                    vt[:, tb * 4 + tt, ts(ob, 512)], ps2[ob][:], AF.Identity
                                )
                            if tb == 0 and tt == 0:
                                # tier-2 issue (see tier-1 note)
                                nc.scalar.dma_start(xq8[:], xq8T[:])
                                nc.scalar.dma_start(wq8_sb[:], wq8T[:])
                    # stage own V tokens (vt[:, 0:8] doubles as projection
                    # scratch; readback rewrites all 16 slots in token order)
                    nc.scalar.dma_start(v_in[:], vt[:, 0:NDT, :])
                    pair_gather(v_in, v_out)

                    # qpos broadcast (fills the gather window)
                    for i in range(4 * CH // 512):
                        bc_ps = pspool.tile([128, 512], F32, name="small", tag="at0", bufs=1)
                        nc.tensor.matmul(
                            bc_ps[:], ones_row[:], qpos_row[:, ts(i, 512)],
                            start=True, stop=True,
                        )
                        nc.scalar.activation(qposB[:, ts(i, 512)], bc_ps[:], AF.Identity)

                    # ---- Q projection: qb pair interleaved under one weight
                    for ot in range(NDT):
                        psq = [
                            pspool.tile([128, 512], F32, name=f"mmq{qb}", tag="mm", bufs=4)
                            for qb in range(2)
                        ]
                        for i2 in range(4):
                            for qb in range(2):
                                nc.tensor.matmul(
                                    psq[qb][:],
                                    wq8_sb[:, i2, :, ts(ot, 128)],
                                    xq8[:, i2, :, ts(qb, 512)],
                                    start=(i2 == 0), stop=(i2 == 3),
                                    perf_mode=DR,
                                )
                        for qb in range(2):
                            nc.scalar.activation(
                                qT8[qb][:, ot // 2, ot % 2, :], psq[qb][:],
                                AF.Identity, bias=bq_sb[:, ot : ot + 1],
                                scale=1.0 / 64.0,
                            )
                        if ot == 0:
                            # tier-3: wo is only needed in P3; issuing it this
                            # late keeps it off the K/V gather's wire window
                            for h in range(2):
                                nc.scalar.dma_start(
                                    wo_sb[:, ts(h, 4), :], woT[:, ts(h, 4), :]
                                )

                # gathered K^T / V readback in true token order, ordered by
                # first use in P2 (sync engine; waits ride on the collectives)
                for r in range(2):
                    nc.sync.dma_start(
                        kT8[:, :, :, r * 1024 + 0 : r * 1024 + 512], ka_out[r]
                    )
                    nc.sync.dma_start(
                        kT8[:, :, :, r * 1024 + 512 : r * 1024 + 1024], kb_out[r]
                    )
                for r in range(2):
                    for h in range(2):
                        nc.sync.dma_start(
                            vt[:, r * 8 + h * 4 : r * 8 + (h + 1) * 4, :],
                            v_out[r, :, ts(h, 4), :],
                        )

                # ---------------- P2: attention ----------------
                # Slots are processed as PAIRS sharing a 512-token q block:
                # the k-range both slots need runs at FD=512 (LDWEIGHTS fully
                # hidden), the hi-slot's excess k-tiles at FD=256.
                # pass A computes scores+exp+mask+denom for all slots first
                # (probs stay resident), so the V gather hides behind it.
                with tc.tile_pool(name="p2", bufs=1) as p2:
                    LOHI = [(NKT[0], NKT[1]), (NKT[2], NKT[3])]
                    pt_sh = [
                        [
                            p2.tile([128, 512], F16, name=f"pts{p}_{k}", tag=f"pts{p}_{k}", bufs=1)
                            for k in range(LOHI[p][0])
                        ]
                        for p in range(2)
                    ]
                    pt_ex = [
                        [
                            p2.tile([128, CH], F16, name=f"ptx{p}_{j}", tag=f"ptx{p}_{j}", bufs=1)
                            for j in range(LOHI[p][1] - LOHI[p][0])
                        ]
                        for p in range(2)
                    ]
                    recipB = [
                        p2.tile([128, CH], F32, name=f"recipB{sl}", tag=f"recipB{sl}", bufs=1)
                        for sl in range(NSLOT)
                    ]
                    for p in range(2):
                        lo, hi = LOHI[p]
                        # shared k-range: FD=512 over both slots
                        for k in range(lo):
                            ps = pspool.tile([128, 512], F32, name="mm", tag="mm", bufs=4)
                            for i2 in range(4):
                                nc.tensor.matmul(
                                    ps[:],
                                    kT8[:, i2, :, ts(k, 128)],
                                    qT8[p][:, i2, :, :],
                                    start=(i2 == 0), stop=(i2 == 3),
                                    perf_mode=DR,
                                )
                            nc.scalar.activation(
                                pt_sh[p][k][:], ps[:], AF.Exp, scale=1.0 / 32.0
                            )
                            if k >= lo - 4:
                                # mask applies to the lo slot's columns only
                                msk = p2.tile([128, CH], F16, name="msk", tag="msk", bufs=2)
                                nc.vector.tensor_scalar(
                                    out=msk[:],
                                    in0=qposB[:, ts(2 * p, CH)],
                                    scalar1=iota_sb[:, k : k + 1],
                                    scalar2=None,
                                    op0=ALU.is_ge,
                                )
                                nc.vector.tensor_tensor(
                                    out=pt_sh[p][k][:, 0:CH],
                                    in0=pt_sh[p][k][:, 0:CH], in1=msk[:],
                                    op=ALU.mult,
                                )
                        # excess k-tiles: hi slot only, FD=256, all masked
                        for j, k in enumerate(range(lo, hi)):
                            ps = pspool.tile([128, CH], F32, name="mm", tag="mm", bufs=4)
                            for i2 in range(4):
                                nc.tensor.matmul(
                                    ps[:],
                                    kT8[:, i2, :, ts(k, 128)],
                                    qT8[p][:, i2, :, CH : 2 * CH],
                                    start=(i2 == 0), stop=(i2 == 3),
                                    perf_mode=DR,
                                )
                            praw = p2.tile([128, CH], F16, name="praw", tag="praw", bufs=2)
                            nc.scalar.activation(
                                praw[:], ps[:], AF.Exp, scale=1.0 / 32.0
                            )
                            msk = p2.tile([128, CH], F16, name="msk", tag="msk", bufs=2)
                            nc.vector.tensor_scalar(
                                out=msk[:],
                                in0=qposB[:, ts(2 * p + 1, CH)],
                                scalar1=iota_sb[:, k : k + 1],
                                scalar2=None,
                                op0=ALU.is_ge,
                            )
                            nc.vector.tensor_tensor(
                                out=pt_ex[p][j][:], in0=praw[:], in1=msk[:],
                                op=ALU.mult,
                            )
                        # denominators per slot (FD=256 chains), then recip
                        for h_ in range(2):
                            sl = 2 * p + h_
                            dn_ps = pspool.tile([1, CH], F32, name="small", tag="at0", bufs=1)
                            nmm = LOHI[p][0] if h_ == 0 else hi
                            kk = 0
                            for k in range(lo):
                                nc.tensor.matmul(
                                    dn_ps[:], ones_col_bf[:],
                                    pt_sh[p][k][:, ts(h_, CH)],
                                    start=(kk == 0), stop=(kk == nmm - 1),
                                )
                                kk += 1
                                if h_ == 0 and kk == nmm:
                                    break
                            if h_ == 1:
                                for j in range(hi - lo):
                                    nc.tensor.matmul(
                                        dn_ps[:], ones_col_bf[:], pt_ex[p][j][:],
                                        start=(kk == 0), stop=(kk == nmm - 1),
                                    )
                                    kk += 1
                            dn_r = p2.tile([1, CH], F32R, name="dn_r", tag="dn_r", bufs=2)
                            nc.vector.tensor_copy(dn_r[:], dn_ps[:])
                            rb_ps = pspool.tile([128, CH], F32, name="small2", tag="at1", bufs=1)
                            nc.tensor.matmul(
                                rb_ps[:], ones_row[:], dn_r[:], start=True, stop=True
                            )
                            nc.vector.reciprocal(recipB[sl][:], rb_ps[:])

                    # pass B: attn^T = (P @ V)^T scaled by 1/denom
                    for p in range(2):
                        lo, hi = LOHI[p]
                        for half in range(2):
                            for d4 in range(4):
                                d_ = half * 4 + d4
                                aps = pspool.tile([128, 512], F32, name=f"at{d4}", tag=f"at{d4}", bufs=1)
                                for k in range(lo):
                                    nc.tensor.matmul(
                                        aps[:],
                                        vt[:, k, ts(d_, 128)],
                                        pt_sh[p][k][:],
                                        start=(k == 0), stop=False,
                                        skip_group_check=True,
                                    )
                                for j, k in enumerate(range(lo, hi)):
                                    nc.tensor.matmul(
                                        aps[:, CH : 2 * CH],
                                        vt[:, k, ts(d_, 128)],
                                        pt_ex[p][j][:],
                                        start=False, stop=(k == hi - 1),
                                        skip_group_check=True,
                                    )
                                if hi == lo:
                                    pass
                                for h_ in range(2):
                                    nc.vector.tensor_tensor(
                                        out=attnT[d_][p][:, ts(h_, CH)],
                                        in0=aps[:, ts(h_, CH)],
                                        in1=recipB[2 * p + h_][:],
                                        op=ALU.mult,
                                    )

                # ---------------- P3 + P4 ----------------
                with tc.tile_pool(name="p34", bufs=1) as p34:
                    outT = [
                        [p34.tile([128, 512], F16, name=f"oT{dt_}_{qb}", tag=f"oT{dt_}_{qb}") for qb in range(2)]
                        for dt_ in range(NDT)
                    ]
                    bfbc = p34.tile([128, 8, 512], F16, name="bfbc", tag="bfbc")
                    for mb in range(M // 512):
                        bc_ps = pspool.tile([128, 512], F32, name="small", tag="at0", bufs=1)
                        nc.tensor.matmul(
                            bc_ps[:], ones_row[:], bfr_sb[:, ts(mb, 512)],
                            start=True, stop=True,
                        )
                        nc.scalar.activation(bfbc[:, mb, :], bc_ps[:], AF.Identity)
                    # P3: qb pair interleaved under one Wo weight block
                    for ot in range(NDT):
                        pso = [
                            pspool.tile([128, 512], F32, name=f"mmo{qb}", tag="mm", bufs=4)
                            for qb in range(2)
                        ]
                        for i in range(NDT):
                            for qb in range(2):
                                nc.tensor.matmul(
                                    pso[qb][:],
                                    wo_sb[:, i, ts(ot, 128)],
                                    attnT[i][qb][:],
                                    start=(i == 0), stop=(i == NDT - 1),
                                )
                        for qb in range(2):
                            nc.scalar.activation(
                                outT[ot][qb][:], pso[qb][:], AF.Identity,
                                bias=bo2_sb[:, ot : ot + 1],
                            )

                    # P4: FFN + GELU. Stationary = outT token-blocks, moving
                    # = the full 512-wide wf block: LDWEIGHTS hides under the
                    # previous matmul (K-pass pattern, 8-bank rotation). The
                    # free-dim bias bf is pre-filled into PSUM by the vector
                    # engine; matmuls accumulate on top (start=False).
                    for mb in range(M // 512):
                        wfb = p34.tile([128, NDT, 512], F16, name="wfb", tag="wfb", bufs=2)
                        nc.gpsimd.dma_start(wfb[:], wfT[mb])
                        st = p34.tile([128, 8, 512], F16, name="ffstage", tag="ffstage", bufs=2)
                        ps8 = [
                            pspool.tile(
                                [128, 512], F32, name=f"ps8_{t8}",
                                tag=(f"at{t8}" if t8 < 4 else "mm"),
                                bufs=(1 if t8 < 4 else 4),
                            )
                            for t8 in range(8)
                        ]
                        for t8 in range(8):
                            qb, tb2 = divmod(t8, 4)
                            nc.vector.tensor_copy(ps8[t8][:], bfbc[:, mb, :])
                            for i in range(NDT):
                                nc.tensor.matmul(
                                    ps8[t8][:],
                                    outT[i][qb][:, ts(tb2, 128)],
                                    wfb[:, i, :],
                                    start=False, stop=(i == NDT - 1),
                                    skip_group_check=True,
                                )
                            nc.scalar.activation(st[:, t8, :], ps8[t8][:], AF.Gelu)
                            if t8 == 3:
                                nc.sync.dma_start(ffT[mb, :, 0:4, :], st[:, 0:4, :])
                        nc.sync.dma_start(ffT[mb, :, 4:8, :], st[:, 4:8, :])

    nc.compile()
    return nc


def _get_program():
    global _PROGRAM
    if _PROGRAM is None:
        _PROGRAM = _build_program()
    return _PROGRAM


def _owned_chunks(core):
    """The four 256-token chunk indices this core owns, in slot order."""
    if core % 2 == 0:
        return (0, 3, 4, 7)
    return (1, 2, 5, 6)


def _blocked(a):
    """[1024, W] -> [128, 8, W] with [p, i, c] = a[i*128+p, c]."""
    W = a.shape[1]
    return np.ascontiguousarray(a.reshape(8, 128, W).transpose(1, 0, 2))


def _pair8(a, scale=1.0):
    """[1024, W] -> [128, 4, 2, W] e4m3 with [p, i2, s, c] = scale*a[(2i2+s)*128+p, c]."""
    W = a.shape[1]
    t = (np.asarray(a, np.float32) * scale).reshape(4, 2, 128, W).transpose(2, 0, 1, 3)
    return np.ascontiguousarray(t).astype(_E4M3)


def _make_in_maps(x, Wq, bq, Wk, bk, Wv, bv, Wo, bo, Wf, bf):
    f32, f16 = np.float32, np.float16
    wq8T = _pair8(np.asarray(Wq.T, np.float32), 64.0)
    wk8T = _pair8(np.asarray(Wk.T, np.float32), 64.0)
    wvT = _blocked(np.asarray(Wv.T, dtype=f16))
    woT = _blocked(np.asarray(Wo.T, dtype=f16))
    # wfT[mb, p, i, c] = Wf.T[i*128+p, mb*512+c]
    wfT = np.ascontiguousarray(
        np.asarray(Wf.T, dtype=f16).reshape(8, 128, 8, 512).transpose(2, 1, 0, 3)
    )
    bo2 = (Wo.astype(np.float64) @ bv.astype(np.float64) + bo.astype(np.float64))
    bo2 = np.ascontiguousarray(bo2.astype(f32).reshape(D // 128, 128).T)
    bfT = np.ascontiguousarray(bf.reshape(M // 128, 128).T, dtype=f32)
    iota = (
        np.arange(128, dtype=f32)[:, None]
        + 128.0 * np.arange(S // 128, dtype=f32)[None, :]
    )
    shared = {
        "wq8T": wq8T, "wk8T": wk8T, "wvT": wvT, "woT": woT, "wfT": wfT,
        "bq": np.ascontiguousarray(bq.reshape(D // 128, 128).T, dtype=f32),
        "bk": np.ascontiguousarray(bk.reshape(D // 128, 128).T, dtype=f32),
        "bo2": bo2,
        "bfT": bfT,
        "bf_row": np.ascontiguousarray(bf[None, :].astype(f32)),
        "iota_kt": np.ascontiguousarray(iota),
    }
    in_maps = []
    for core in range(N_CORES):
        b = core // 2
        chunks = _owned_chunks(core)
        xTb = np.asarray(x[b].T, dtype=f16)  # [D, S]
        half = core % 2  # rank within the pair: rank0 owns tokens 0:S/2
        xown = xTb[:, half * (S // 2) : (half + 1) * (S // 2)]
        xaT = _blocked(xown)
        xa8T = _pair8(xown.astype(np.float32))
        xq8T = _pair8(
            np.concatenate(
                [xTb[:, c * CH : (c + 1) * CH] for c in chunks], axis=1
            ).astype(np.float32)
        )
        qp = np.concatenate(
            [np.arange(c * CH, (c + 1) * CH) for c in chunks]
        ).astype(f32)[None, :]
        in_maps.append(
            {**shared, "xaT": xaT, "xa8T": xa8T, "xq8T": xq8T,
             "qpos": np.ascontiguousarray(qp)}
        )
    return in_maps


def _run(inputs, trace=False, trace_cores=None, tmpdir=None):
    import sys

    if "/opt/trn_rl_repo" not in sys.path:
        sys.path.insert(0, "/opt/trn_rl_repo")
    from concourse.bass_utils import run_bass_kernel_spmd

    nc = _get_program()
    in_maps = _make_in_maps(**inputs)
    res = run_bass_kernel_spmd(
        nc, in_maps, list(range(N_CORES)), trace=trace, trace_cores=trace_cores,
        tmpdir=tmpdir,
    )
    out = np.empty((B, S, M), dtype=np.float32)
    for core in range(N_CORES):
        b = core // 2
        chunks = _owned_chunks(core)
        # ffT[mb, p, t8, c] = ff[(t8//4)*512 + (t8%4)*128 + p, mb*512 + c]
        raw = res.results[core]["ffT"].reshape(8, 128, 2, 4, 512)
        ff = np.ascontiguousarray(
            raw.transpose(2, 3, 1, 0, 4)
        ).reshape(4 * CH, M)
        for sl, c in enumerate(chunks):
            qb, qo = divmod(sl, 2)
            out[b, c * CH : (c + 1) * CH] = (
                ff[qb * 512 + qo * CH : qb * 512 + (qo + 1) * CH].astype(np.float32)
            )
    return out, res


def kernel(**inputs):
    out, _ = _run(inputs)
    return out


# revision 25
# speedup vs baseline: 1.1176x; 1.1176x over previous
"""Decoder block (single-head causal attention + GELU FFN) on 8 TRN2 NeuronCores.

Sharding: data parallel over batch (2 cores per batch), with the K AND V
projections token-split across the pair (each core projects its own half of
the sequence; two pairwise AllGathers share K, one shares V). Core c handles
batch b = c//2 and 1024 query tokens of that batch, chosen as four 256-token
chunks that balance the causal-attention workload:
  even cores (half 0): chunks 0, 3, 4, 7
  odd  cores (half 1): chunks 1, 2, 5, 6
The slot pairing makes the static k-tile counts per slot (4, 8, 12, 16) cover
both cores' needs with minimal waste; the gap is zeroed by the data-driven
qpos mask. The SPMD program is identical on every core.

Performance structure (final):
  - Q/K projections and QK^T scores run in fp8 e4m3 with DoubleRow perf mode
    (2 contraction rows per PE cell). QK weights are host-scaled by 64 (so
    they clear e4m3's subnormal cutoff) and the projection drains rescale by
    1/64. Q^T/K^T are written fp8 directly at their PSUM drains; the K
    gathers move fp8 (half the payload). Measured end-to-end rel err 1.5e-2
    vs the 2e-2 gate; V/PV/Wo/FFN stay fp16 (fp8 there fails the gate).
  - P2 processes slots as pairs sharing a 512-token q block: the shared
    k-range runs at FD=512 (LDWEIGHTS fully hidden), the hi slot's excess
    k-tiles at FD=256. All scores+softmax run first (pass A), all PV second
    (pass B), so the V-gather latency hides behind pass A.
  - The collective chain cannot start transfers before ~60us regardless of
    trigger time (channel init), so K gathers are split in two 0.5MB halves
    (first halves arrive in time for pass A) while V is one 2MB gather that
    finishes during pass A.
  - P4 makes the 512-wide wf block the MOVING operand (stationary = outT
    token-blocks, t8-outer/i-inner over all 8 PSUM banks): streams at
    ~213ns/matmul with LDWEIGHTS fully hidden. The free-dim FFN bias is
    pre-filled into PSUM by the vector engine (matmuls use start=False);
    GELU drains need no bias.
  - Q-proj/P3 interleave the two 512-token column blocks under one
    stationary weight load; K-pass hides LDWEIGHTS via 4-bank rotation.
  - DMA issue is tiered by first-use (wk8/xa8 at t0 on gpsimd/sync, wv/xa on
    scalar behind tiny consts, xq8/wq8 after V starts, wo after Q starts) so
    concurrent streams never starve the critical path; all loads are few big
    host-pre-arranged contiguous transfers.
  - scalar runs only Identity in P1 and only Exp in P2 (activation table
    reloads cost 1.3us); stores go from sync; biases fold: bo2 = Wo@bv + bo.
"""

import numpy as np
import ml_dtypes

_E4M3 = ml_dtypes.float8_e4m3fn

D = 1024  # model dim
S = 2048  # sequence length
B = 4  # batch
M = 4096  # FFN dim
CH = 256  # q chunk (slot) size
NSLOT = 4  # q slots per core
NDT = D // 128  # 8 d-tiles
N_CORES = 8
NKT = [4, 8, 12, 16]  # k-tiles per slot (static max over the two paired cores)

_PROGRAM = None  # cached compiled program


def _build_program():
    import sys

    if "/opt/trn_rl_repo" not in sys.path:
        sys.path.insert(0, "/opt/trn_rl_repo")
    import concourse.bass as bass
    import concourse.tile as tile
    import concourse.mybir as mybir
    from concourse import bacc
    from concourse.bass import ts

    dt = mybir.dt
    AF = mybir.ActivationFunctionType
    ALU = mybir.AluOpType
    F32, F32R, F16, F8 = dt.float32, dt.float32r, dt.float16, dt.float8e4
    DR = mybir.MatmulPerfMode.DoubleRow

    nc = bacc.Bacc("TRN2", target_bir_lowering=False, debug=False, num_devices=8)

    # ---------------- DRAM I/O (all host-pre-arranged layouts) ----------------
    # fp16 weights: [128, i(8), 1024] with [p, i, c] = W.T[i*128+p, c]
    wvT = nc.dram_tensor("wvT", [128, NDT, D], F16, kind="ExternalInput").ap()
    woT = nc.dram_tensor("woT", [128, NDT, D], F16, kind="ExternalInput").ap()
    # fp8 QK weights in DoubleRow pair layout, host-scaled by 64:
    # [p, i2, s, c] = e4m3(64 * W.T[(2*i2+s)*128+p, c])
    wq8T = nc.dram_tensor("wq8T", [128, 4, 2, D], F8, kind="ExternalInput").ap()
    wk8T = nc.dram_tensor("wk8T", [128, 4, 2, D], F8, kind="ExternalInput").ap()
    # FFN weight: [mb(8), 128, i(8), 512] with [mb, p, i, c] = Wf.T[i*128+p, mb*512+c]
    wfT = nc.dram_tensor("wfT", [M // 512, 128, NDT, 512], F16, kind="ExternalInput").ap()
    # x, own-half tokens in k order: [p, i, t] = x.T[i*128+p, half*1024+t]
    xaT = nc.dram_tensor("xaT", [128, NDT, S // 2], F16, kind="ExternalInput").ap()
    # fp8 x copies in DoubleRow pair layout (for the Q/K projections)
    xa8T = nc.dram_tensor("xa8T", [128, 4, 2, S // 2], F8, kind="ExternalInput").ap()
    xq8T = nc.dram_tensor("xq8T", [128, 4, 2, 4 * CH], F8, kind="ExternalInput").ap()
    bq = nc.dram_tensor("bq", [128, D // 128], F32, kind="ExternalInput").ap()
    bk = nc.dram_tensor("bk", [128, D // 128], F32, kind="ExternalInput").ap()
    bo2 = nc.dram_tensor("bo2", [128, D // 128], F32, kind="ExternalInput").ap()
    bfT = nc.dram_tensor("bfT", [128, M // 128], F32, kind="ExternalInput").ap()
    qpos = nc.dram_tensor("qpos", [1, 4 * CH], F32R, kind="ExternalInput").ap()
    bf_row = nc.dram_tensor("bf_row", [1, M], F32R, kind="ExternalInput").ap()
    iota_kt = nc.dram_tensor("iota_kt", [128, S // 128], F32, kind="ExternalInput").ap()
    # output: [mb(8), 128p, t8(8), 512c] = ff[(t8//4)*512+(t8%4)*128+p, mb*512+c]
    ffT = nc.dram_tensor("ffT", [M // 512, 128, 8, 512], F16, kind="ExternalOutput").ap()

    with tile.TileContext(nc) as tc:
        with (
            tc.tile_pool(name="const", bufs=1) as cpool,
            tc.tile_pool(name="psum", bufs=1, space="PSUM") as pspool,
        ):
            # ---------------- constants (scalar engine issues these) --------
            ones_col_bf = cpool.tile([128, 1], F16, name="ones_col_bf", tag="ones_col_bf")
            nc.vector.memset(ones_col_bf[:], 1.0)
            ones_row_f = cpool.tile([1, 128], F32, name="ones_row_f", tag="ones_row_f")
            nc.vector.memset(ones_row_f[:], 1.0)
            ones_row = cpool.tile([1, 128], F32R, name="ones_row", tag="ones_row")
            nc.vector.tensor_copy(ones_row[:], ones_row_f[:])
            iota_sb = cpool.tile([128, S // 128], F32, name="iota", tag="iota")
            nc.scalar.dma_start(iota_sb[:], iota_kt[:])
            bq_sb = cpool.tile([128, D // 128], F32, name="bq", tag="bq")
            nc.scalar.dma_start(bq_sb[:], bq[:])
            bk_sb = cpool.tile([128, D // 128], F32, name="bk", tag="bk")
            nc.scalar.dma_start(bk_sb[:], bk[:])
            bo2_sb = cpool.tile([128, D // 128], F32, name="bo2", tag="bo2")
            nc.scalar.dma_start(bo2_sb[:], bo2[:])
            bf_sb = cpool.tile([128, M // 128], F32, name="bf", tag="bf")
            nc.scalar.dma_start(bf_sb[:], bfT[:])
            qpos_row = cpool.tile([1, 4 * CH], F32R, name="qpos_row", tag="qpos_row")
            nc.scalar.dma_start(qpos_row[:], qpos[:])
            bfr_sb = cpool.tile([1, M], F32R, name="bfr_sb", tag="bfr_sb")
            nc.scalar.dma_start(bfr_sb[:], bf_row[:])
            qposB = cpool.tile([128, 4 * CH], F32, name="qposB", tag="qposB")
            # V-pass inputs stream from scalar's queue at t0: they follow the
            # (tiny) const DMAs, so the critical wk8/xa8 pieces on gpsimd/sync
            # still see most of the wire, and wv/xa are in well before V needs
            # them (~25us)

            # ------------- long-lived tiles: one pool spanning P1..P4 -------
            with (
                tc.tile_pool(name="main", bufs=1) as mp,
                tc.tile_pool(name="dram", bufs=1, space="DRAM") as dram,
            ):
                # fp8 K^T in DoubleRow pair layout: [p, i2, s, tok],
                # contraction d = (2*i2+s)*128 + p
                kT8 = mp.tile([128, 4, 2, S], F8, name="kT8", tag="kT8")
                vt = mp.tile([128, 16, D], F16, name="vt", tag="vt")
                wo_sb = mp.tile([128, NDT, D], F16, name="wo", tag="wo")
                qT8 = [
                    mp.tile([128, 4, 2, 512], F8, name=f"qT8_{qb}", tag=f"qT8_{qb}")
                    for qb in range(2)
                ]
                attnT = [
                    [mp.tile([128, 512], F16, name=f"at{dt_}_{qb}", tag=f"at{dt_}_{qb}") for qb in range(2)]
                    for dt_ in range(NDT)
                ]
                # DRAM bounce buffers for the pairwise K and V AllGathers.
                # Each projection is gathered in two 1MB halves so the
                # collectives fire earlier and finish well before P2 needs
                # the peer's tokens.
                ka_in = dram.tile([128, 4, 2, 512], F8, name="ka_in", tag="ka_in")
                ka_out = dram.tile([2, 128, 4, 2, 512], F8, name="ka_out", tag="ka_out")
                kb_in = dram.tile([128, 4, 2, 512], F8, name="kb_in", tag="kb_in")
                kb_out = dram.tile([2, 128, 4, 2, 512], F8, name="kb_out", tag="kb_out")
                v_in = dram.tile([128, NDT, D], F16, name="v_in", tag="v_in")
                v_out = dram.tile([2, 128, NDT, D], F16, name="v_out", tag="v_out")

                def pair_gather(in_t, out_t):
                    nc.gpsimd.collective_compute(
                        "AllGather",
                        mybir.AluOpType.bypass,
                        replica_groups=[[0, 1], [2, 3], [4, 5], [6, 7]],
                        ins=[in_t[:].opt()],
                        outs=[out_t[:].opt()],
                    )

                # ---------------- P1 ----------------
                with tc.tile_pool(name="p1a", bufs=1) as p1a:
                    wk8_sb = p1a.tile([128, 4, 2, D], F8, name="wk8", tag="wk8")
                    wq8_sb = p1a.tile([128, 4, 2, D], F8, name="wq8", tag="wq8")
                    wv_sb = p1a.tile([128, NDT, D], F16, name="wv", tag="wv")
                    xa = p1a.tile([128, NDT, S // 2], F16, name="xa", tag="xa")
                    xa8 = p1a.tile([128, 4, 2, S // 2], F8, name="xa8", tag="xa8")
                    xq8 = p1a.tile([128, 4, 2, 4 * CH], F8, name="xq8", tag="xq8")
                    for h in range(2):
                        nc.scalar.dma_start(
                            wv_sb[:, ts(h, 4), :], wvT[:, ts(h, 4), :]
                        )
                        nc.scalar.dma_start(
                            xa[:, ts(h, 4), :], xaT[:, ts(h, 4), :]
                        )

                    # DMA issue is tiered: all in-flight DMAs share wire
                    # bandwidth, so only the critical-path wk/xa stream starts
                    # immediately (per-i granularity: compute starts on the
                    # first 256KB). Later streams are gated on compute
                    # sentinels and issued from the otherwise-idle vector
                    # engine so they cannot steal bandwidth early.
                    for h in range(2):
                        nc.gpsimd.dma_start(
                            wk8_sb[:, :, :, ts(h, 512)], wk8T[:, :, :, ts(h, 512)]
                        )
                        nc.sync.dma_start(
                            xa8[:, :, :, ts(h, 512)], xa8T[:, :, :, ts(h, 512)]
                        )


                    # ---- K projection over own half (kT staged at [:, :, 0:1024])
                    for tb in range(2):
                        for og in range(2):
                            ps4 = [
                                pspool.tile([128, 512], F32, name=f"at{j}", tag=f"at{j}", bufs=1)
                                for j in range(4)
                            ]
                            for i2 in range(4):
                                for j in range(4):
                                    nc.tensor.matmul(
                                        ps4[j][:],
                                        wk8_sb[:, i2, :, ts(og * 4 + j, 128)],
                                        xa8[:, i2, :, ts(tb, 512)],
                                        start=(i2 == 0), stop=(i2 == 3),
                                        perf_mode=DR,
                                    )
                            for j in range(4):
                                ot = og * 4 + j
                                nc.scalar.activation(
                                    kT8[:, ot // 2, ot % 2, ts(tb, 512)], ps4[j][:],
                                    AF.Identity, bias=bk_sb[:, ot : ot + 1],
                                    scale=1.0 / 64.0,
                                )
                            if tb == 0 and og == 0:
                                pass
                        # stage this token block and fire its K gather
                        kin = ka_in if tb == 0 else kb_in
                        nc.sync.dma_start(kin[:], kT8[:, :, :, ts(tb, 512)])
                        pair_gather(kin, ka_out if tb == 0 else kb_out)

                    # ---- V projection over own half (token-major, 2 banks/tt)
                    for tb in range(2):
                        for tt in range(4):
                            ps2 = [
                                pspool.tile([128, 512], F32, name=f"mm{ob}", tag="mm", bufs=4)
                                for ob in range(2)
                            ]
                            for i in range(NDT):
                                for ob in range(2):
                                    nc.tensor.matmul(
                                        ps2[ob][:],
                                        xa[:, i, tb * 512 + tt * 128 : tb * 512 + (tt + 1) * 128],
                                        wv_sb[:, i, ts(ob, 512)],
                                        start=(i == 0), stop=(i == NDT - 1),
                                    )
                            for ob in range(2):
                                nc.scalar.activation(
                                    vt[:, tb * 4 + tt, ts(ob, 512)], ps2[ob][:], AF.Identity
                                )
                            if tb == 0 and tt == 0:
                                # tier-2 issue (see tier-1 note)
                                nc.scalar.dma_start(xq8[:], xq8T[:])
                                nc.scalar.dma_start(wq8_sb[:], wq8T[:])
                    # stage own V tokens (vt[:, 0:8] doubles as projection
                    # scratch; readback rewrites all 16 slots in token order)
                    nc.scalar.dma_start(v_in[:], vt[:, 0:NDT, :])
                    pair_gather(v_in, v_out)

                    # qpos broadcast (fills the gather window)
                    for i in range(4 * CH // 512):
                        bc_ps = pspool.tile([128, 512], F32, name="small", tag="at0", bufs=1)
                        nc.tensor.matmul(
                            bc_ps[:], ones_row[:], qpos_row[:, ts(i, 512)],
                            start=True, stop=True,
                        )
                        nc.scalar.activation(qposB[:, ts(i, 512)], bc_ps[:], AF.Identity)

                    # ---- Q projection: qb pair interleaved under one weight
                    for ot in range(NDT):
                        psq = [
                            pspool.tile([128, 512], F32, name=f"mmq{qb}", tag="mm", bufs=4)
                            for qb in range(2)
                        ]
                        for i2 in range(4):
                            for qb in range(2):
                                nc.tensor.matmul(
                                    psq[qb][:],
                                    wq8_sb[:, i2, :, ts(ot, 128)],
                                    xq8[:, i2, :, ts(qb, 512)],
                                    start=(i2 == 0), stop=(i2 == 3),
                                    perf_mode=DR,
                                )
                        for qb in range(2):
                            nc.scalar.activation(
                                qT8[qb][:, ot // 2, ot % 2, :], psq[qb][:],
                                AF.Identity, bias=bq_sb[:, ot : ot + 1],
                                scale=1.0 / 64.0,
                            )
                        if ot == 0:
                            # tier-3: wo is only needed in P3; issuing it this
                            # late keeps it off the K/V gather's wire window
                            for h in range(2):
                                nc.scalar.dma_start(
                                    wo_sb[:, ts(h, 4), :], woT[:, ts(h, 4), :]
                                )

                # gathered K^T / V readback in true token order, ordered by
                # first use in P2 (sync engine; waits ride on the collectives)
                for r in range(2):
                    nc.sync.dma_start(
                        kT8[:, :, :, r * 1024 + 0 : r * 1024 + 512], ka_out[r]
                    )
                    nc.sync.dma_start(
                        kT8[:, :, :, r * 1024 + 512 : r * 1024 + 1024], kb_out[r]
                    )
                for r in range(2):
                    for h in range(2):
                        nc.sync.dma_start(
                            vt[:, r * 8 + h * 4 : r * 8 + (h + 1) * 4, :],
                            v_out[r, :, ts(h, 4), :],
                        )

                # ---------------- P2: attention ----------------
                # Slots are processed as PAIRS sharing a 512-token q block:
                # the k-range both slots need runs at FD=512 (LDWEIGHTS fully
                # hidden), the hi-slot's excess k-tiles at FD=256.
                # pass A computes scores+exp+mask+denom for all slots first
                # (probs stay resident), so the V gather hides behind it.
                with tc.tile_pool(name="p2", bufs=1) as p2:
                    LOHI = [(NKT[0], NKT[1]), (NKT[2], NKT[3])]
                    pt_sh = [
                        [
                            p2.tile([128, 512], F16, name=f"pts{p}_{k}", tag=f"pts{p}_{k}", bufs=1)
                            for k in range(LOHI[p][0])
                        ]
                        for p in range(2)
                    ]
                    pt_ex = [
                        [
                            p2.tile([128, CH], F16, name=f"ptx{p}_{j}", tag=f"ptx{p}_{j}", bufs=1)
                            for j in range(LOHI[p][1] - LOHI[p][0])
                        ]
                        for p in range(2)
                    ]
                    recipB = [
                        p2.tile([128, CH], F32, name=f"recipB{sl}", tag=f"recipB{sl}", bufs=1)
                        for sl in range(NSLOT)
                    ]
                    for p in range(2):
                        lo, hi = LOHI[p]
                        # shared k-range: FD=512 over both slots
                        for k in range(lo):
                            ps = pspool.tile([128, 512], F32, name="mm", tag="mm", bufs=4)
                            for i2 in range(4):
                                nc.tensor.matmul(
                                    ps[:],
                                    kT8[:, i2, :, ts(k, 128)],
                                    qT8[p][:, i2, :, :],
                                    start=(i2 == 0), stop=(i2 == 3),
                                    perf_mode=DR,
                                )
                            nc.scalar.activation(
                                pt_sh[p][k][:], ps[:], AF.Exp, scale=1.0 / 32.0
                            )
                            if k >= lo - 4:
                                # mask applies to the lo slot's columns only
                                msk = p2.tile([128, CH], F16, name="msk", tag="msk", bufs=2)
                                nc.vector.tensor_scalar(
                                    out=msk[:],
                                    in0=qposB[:, ts(2 * p, CH)],
                                    scalar1=iota_sb[:, k : k + 1],
                                    scalar2=None,
                                    op0=ALU.is_ge,
                                )
                                nc.vector.tensor_tensor(
                                    out=pt_sh[p][k][:, 0:CH],
                                    in0=pt_sh[p][k][:, 0:CH], in1=msk[:],
                                    op=ALU.mult,
                                )
                        # excess k-tiles: hi slot only, FD=256, all masked
                        for j, k in enumerate(range(lo, hi)):
                            ps = pspool.tile([128, CH], F32, name="mm", tag="mm", bufs=4)
                            for i2 in range(4):
                                nc.tensor.matmul(
                                    ps[:],
                                    kT8[:, i2, :, ts(k, 128)],
                                    qT8[p][:, i2, :, CH : 2 * CH],
                                    start=(i2 == 0), stop=(i2 == 3),
                                    perf_mode=DR,
                                )
                            praw = p2.tile([128, CH], F16, name="praw", tag="praw", bufs=2)
                            nc.scalar.activation(
                                praw[:], ps[:], AF.Exp, scale=1.0 / 32.0
                            )
                            msk = p2.tile([128, CH], F16, name="msk", tag="msk", bufs=2)
                            nc.vector.tensor_scalar(
                                out=msk[:],
                                in0=qposB[:, ts(2 * p + 1, CH)],
                                scalar1=iota_sb[:, k : k + 1],
                                scalar2=None,
                                op0=ALU.is_ge,
                            )
                            nc.vector.tensor_tensor(
                                out=pt_ex[p][j][:], in0=praw[:], in1=msk[:],
                                op=ALU.mult,
                            )
                        # denominators per slot (FD=256 chains), then recip
                        for h_ in range(2):
                            sl = 2 * p + h_
                            dn_ps = pspool.tile([1, CH], F32, name="small", tag="at0", bufs=1)
                            nmm = LOHI[p][0] if h_ == 0 else hi
                            kk = 0
                            for k in range(lo):
                                nc.tensor.matmul(
                                    dn_ps[:], ones_col_bf[:],
                                    pt_sh[p][k][:, ts(h_, CH)],
                                    start=(kk == 0), stop=(kk == nmm - 1),
                                )
                                kk += 1
                                if h_ == 0 and kk == nmm:
                                    break
                            if h_ == 1:
                                for j in range(hi - lo):
                                    nc.tensor.matmul(
                                        dn_ps[:], ones_col_bf[:], pt_ex[p][j][:],
                                        start=(kk == 0), stop=(kk == nmm - 1),
                                    )
                                    kk += 1
                            dn_r = p2.tile([1, CH], F32R, name="dn_r", tag="dn_r", bufs=2)
                            nc.vector.tensor_copy(dn_r[:], dn_ps[:])
                            rb_ps = pspool.tile([128, CH], F32, name="small2", tag="at1", bufs=1)
                            nc.tensor.matmul(
                                rb_ps[:], ones_row[:], dn_r[:], start=True, stop=True
                            )
                            nc.vector.reciprocal(recipB[sl][:], rb_ps[:])

                    # pass B: attn^T = (P @ V)^T scaled by 1/denom
                    for p in range(2):
                        lo, hi = LOHI[p]
                        for half in range(2):
                            for d4 in range(4):
                                d_ = half * 4 + d4
                                aps = pspool.tile([128, 512], F32, name=f"at{d4}", tag=f"at{d4}", bufs=1)
                                for k in range(lo):
                                    nc.tensor.matmul(
                                        aps[:],
                                        vt[:, k, ts(d_, 128)],
                                        pt_sh[p][k][:],
                                        start=(k == 0), stop=False,
                                        skip_group_check=True,
                                    )
                                for j, k in enumerate(range(lo, hi)):
                                    nc.tensor.matmul(
                                        aps[:, CH : 2 * CH],
                                        vt[:, k, ts(d_, 128)],
                                        pt_ex[p][j][:],
                                        start=False, stop=(k == hi - 1),
                                        skip_group_check=True,
                                    )
                                if hi == lo:
                                    pass
                                for h_ in range(2):
                                    nc.vector.tensor_tensor(
                                        out=attnT[d_][p][:, ts(h_, CH)],
                                        in0=aps[:, ts(h_, CH)],
                                        in1=recipB[2 * p + h_][:],
                                        op=ALU.mult,
                                    )

                # ---------------- P3 + P4 ----------------
                with tc.tile_pool(name="p34", bufs=1) as p34:
                    outT = [
                        [p34.tile([128, 512], F16, name=f"oT{dt_}_{qb}", tag=f"oT{dt_}_{qb}") for qb in range(2)]
                        for dt_ in range(NDT)
                    ]
                    bfbc = p34.tile([128, 8, 512], F16, name="bfbc", tag="bfbc")
                    for mb in range(M // 512):
                        bc_ps = pspool.tile([128, 512], F32, name="small", tag="at0", bufs=1)
                        nc.tensor.matmul(
                            bc_ps[:], ones_row[:], bfr_sb[:, ts(mb, 512)],
                            start=True, stop=True,
                        )
                        nc.scalar.activation(bfbc[:, mb, :], bc_ps[:], AF.Identity)
                    # P3: qb pair interleaved under one Wo weight block
                    for ot in range(NDT):
                        pso = [
                            pspool.tile([128, 512], F32, name=f"mmo{qb}", tag="mm", bufs=4)
                            for qb in range(2)
                        ]
                        for i in range(NDT):
                            for qb in range(2):
                                nc.tensor.matmul(
                                    pso[qb][:],
                                    wo_sb[:, i, ts(ot, 128)],
                                    attnT[i][qb][:],
                                    start=(i == 0), stop=(i == NDT - 1),
                                )
                        for qb in range(2):
                            nc.scalar.activation(
                                outT[ot][qb][:], pso[qb][:], AF.Identity,
                                bias=bo2_sb[:, ot : ot + 1],
                            )

                    # P4: FFN + GELU. Stationary = outT token-blocks, moving
                    # = the full 512-wide wf block: LDWEIGHTS hides under the
                    # previous matmul (K-pass pattern, 8-bank rotation). The
                    # free-dim bias bf is pre-filled into PSUM by the vector
                    # engine; matmuls accumulate on top (start=False).
                    for mb in range(M // 512):
                        wfb = p34.tile([128, NDT, 512], F16, name="wfb", tag="wfb", bufs=2)
                        nc.gpsimd.dma_start(wfb[:], wfT[mb])
                        st = p34.tile([128, 8, 512], F16, name="ffstage", tag="ffstage", bufs=2)
                        ps8 = [
                            pspool.tile(
                                [128, 512], F32, name=f"ps8_{t8}",
                                tag=(f"at{t8}" if t8 < 4 else "mm"),
                                bufs=(1 if t8 < 4 else 4),
                            )
                            for t8 in range(8)
                        ]
                        for t8 in range(8):
                            qb, tb2 = divmod(t8, 4)
                            nc.vector.tensor_copy(ps8[t8][:], bfbc[:, mb, :])
                            for i in range(NDT):
                                nc.tensor.matmul(
                                    ps8[t8][:],
                                    outT[i][qb][:, ts(tb2, 128)],
                                    wfb[:, i, :],
                                    start=False, stop=(i == NDT - 1),
                                    skip_group_check=True,
                                )
                            nc.scalar.activation(st[:, t8, :], ps8[t8][:], AF.Gelu)
                            if t8 == 3:
                                nc.sync.dma_start(ffT[mb, :, 0:4, :], st[:, 0:4, :])
                        nc.sync.dma_start(ffT[mb, :, 4:8, :], st[:, 4:8, :])

    nc.compile()
    return nc


def _get_program():
    global _PROGRAM
    if _PROGRAM is None:
        _PROGRAM = _build_program()
    return _PROGRAM


def _owned_chunks(core):
    """The four 256-token chunk indices this core owns, in slot order."""
    if core % 2 == 0:
        return (0, 3, 4, 7)
    return (1, 2, 5, 6)


def _blocked(a):
    """[1024, W] -> [128, 8, W] with [p, i, c] = a[i*128+p, c]."""
    W = a.shape[1]
    return np.ascontiguousarray(a.reshape(8, 128, W).transpose(1, 0, 2))


def _pair8(a, scale=1.0):
    """[1024, W] -> [128, 4, 2, W] e4m3 with [p, i2, s, c] = scale*a[(2i2+s)*128+p, c]."""
    W = a.shape[1]
    t = (np.asarray(a, np.float32) * scale).reshape(4, 2, 128, W).transpose(2, 0, 1, 3)
    return np.ascontiguousarray(t).astype(_E4M3)


def _make_in_maps(x, Wq, bq, Wk, bk, Wv, bv, Wo, bo, Wf, bf):
    f32, f16 = np.float32, np.float16
    wq8T = _pair8(np.asarray(Wq.T, np.float32), 64.0)
    wk8T = _pair8(np.asarray(Wk.T, np.float32), 64.0)
    wvT = _blocked(np.asarray(Wv.T, dtype=f16))
    woT = _blocked(np.asarray(Wo.T, dtype=f16))
    # wfT[mb, p, i, c] = Wf.T[i*128+p, mb*512+c]
    wfT = np.ascontiguousarray(
        np.asarray(Wf.T, dtype=f16).reshape(8, 128, 8, 512).transpose(2, 1, 0, 3)
    )
    bo2 = (Wo.astype(np.float64) @ bv.astype(np.float64) + bo.astype(np.float64))
    bo2 = np.ascontiguousarray(bo2.astype(f32).reshape(D // 128, 128).T)
    bfT = np.ascontiguousarray(bf.reshape(M // 128, 128).T, dtype=f32)
    iota = (
        np.arange(128, dtype=f32)[:, None]
        + 128.0 * np.arange(S // 128, dtype=f32)[None, :]
    )
    shared = {
        "wq8T": wq8T, "wk8T": wk8T, "wvT": wvT, "woT": woT, "wfT": wfT,
        "bq": np.ascontiguousarray(bq.reshape(D // 128, 128).T, dtype=f32),
        "bk": np.ascontiguousarray(bk.reshape(D // 128, 128).T, dtype=f32),
        "bo2": bo2,
        "bfT": bfT,
        "bf_row": np.ascontiguousarray(bf[None, :].astype(f32)),
        "iota_kt": np.ascontiguousarray(iota),
    }
    in_maps = []
    for core in range(N_CORES):
        b = core // 2
        chunks = _owned_chunks(core)
        xTb = np.asarray(x[b].T, dtype=f16)  # [D, S]
        half = core % 2  # rank within the pair: rank0 owns tokens 0:S/2
        xown = xTb[:, half * (S // 2) : (half + 1) * (S // 2)]
        xaT = _blocked(xown)
        xa8T = _pair8(xown.astype(np.float32))
        xq8T = _pair8(
            np.concatenate(
                [xTb[:, c * CH : (c + 1) * CH] for c in chunks], axis=1
            ).astype(np.float32)
        )
        qp = np.concatenate(
            [np.arange(c * CH, (c + 1) * CH) for c in chunks]
        ).astype(f32)[None, :]
        in_maps.append(
            {**shared, "xaT": xaT, "xa8T": xa8T, "xq8T": xq8T,
             "qpos": np.ascontiguousarray(qp)}
        )
    return in_maps


def _run(inputs, trace=False, trace_cores=None, tmpdir=None):
    import sys

    if "/opt/trn_rl_repo" not in sys.path:
        sys.path.insert(0, "/opt/trn_rl_repo")
    from concourse.bass_utils import run_bass_kernel_spmd

    nc = _get_program()
    in_maps = _make_in_maps(**inputs)
    res = run_bass_kernel_spmd(
        nc, in_maps, list(range(N_CORES)), trace=trace, trace_cores=trace_cores,
        tmpdir=tmpdir,
    )
    out = np.empty((B, S, M), dtype=np.float32)
    for core in range(N_CORES):
        b = core // 2
        chunks = _owned_chunks(core)
        # ffT[mb, p, t8, c] = ff[(t8//4)*512 + (t8%4)*128 + p, mb*512 + c]
        raw = res.results[core]["ffT"].reshape(8, 128, 2, 4, 512)
        ff = np.ascontiguousarray(
            raw.transpose(2, 3, 1, 0, 4)
        ).reshape(4 * CH, M)
        for sl, c in enumerate(chunks):
            qb, qo = divmod(sl, 2)
            out[b, c * CH : (c + 1) * CH] = (
                ff[qb * 512 + qo * CH : qb * 512 + (qo + 1) * CH].astype(np.float32)
            )
    return out, res


def kernel(**inputs):
    out, _ = _run(inputs)
    return out


# revision 26
# speedup vs baseline: 1.1443x; 1.0239x over previous
"""Decoder block (single-head causal attention + GELU FFN) on 8 TRN2 NeuronCores.

Sharding: data parallel over batch (2 cores per batch), with the K AND V
projections token-split across the pair (each core projects its own half of
the sequence; two pairwise AllGathers share K, one shares V). Core c handles
batch b = c//2 and 1024 query tokens of that batch, chosen as four 256-token
chunks that balance the causal-attention workload:
  even cores (half 0): chunks 0, 3, 4, 7
  odd  cores (half 1): chunks 1, 2, 5, 6
The slot pairing makes the static k-tile counts per slot (4, 8, 12, 16) cover
both cores' needs with minimal waste; the gap is zeroed by the data-driven
qpos mask. The SPMD program is identical on every core.

Performance structure (final):
  - Q/K projections and QK^T scores run in fp8 e4m3 with DoubleRow perf mode
    (2 contraction rows per PE cell). QK weights are host-scaled by 64 (so
    they clear e4m3's subnormal cutoff) and the projection drains rescale by
    1/64. Q^T/K^T are written fp8 directly at their PSUM drains; the K
    gathers move fp8 (half the payload). Measured end-to-end rel err 1.5e-2
    vs the 2e-2 gate; V/PV/Wo/FFN stay fp16 (fp8 there fails the gate).
  - P2 processes slots as pairs sharing a 512-token q block: the shared
    k-range runs at FD=512 (LDWEIGHTS fully hidden), the hi slot's excess
    k-tiles at FD=256. All scores+softmax run first (pass A), all PV second
    (pass B), so the V-gather latency hides behind pass A.
  - The collective chain cannot start transfers before ~60us regardless of
    trigger time (channel init), so K gathers are split in two 0.5MB halves
    (first halves arrive in time for pass A) while V is one 2MB gather that
    finishes during pass A.
  - P4 makes the 512-wide wf block the MOVING operand (stationary = outT
    token-blocks, t8-outer/i-inner over all 8 PSUM banks): streams at
    ~213ns/matmul with LDWEIGHTS fully hidden. The free-dim FFN bias is
    pre-filled into PSUM by the vector engine (matmuls use start=False);
    GELU drains need no bias.
  - Q-proj/P3 interleave the two 512-token column blocks under one
    stationary weight load; K-pass hides LDWEIGHTS via 4-bank rotation.
  - DMA issue is tiered by first-use (wk8/xa8 at t0 on gpsimd/sync, wv/xa on
    scalar behind tiny consts, xq8/wq8 after V starts, wo after Q starts) so
    concurrent streams never starve the critical path; all loads are few big
    host-pre-arranged contiguous transfers.
  - scalar runs only Identity in P1 and only Exp in P2 (activation table
    reloads cost 1.3us); stores go from sync; biases fold: bo2 = Wo@bv + bo.
"""

import numpy as np
import ml_dtypes

_E4M3 = ml_dtypes.float8_e4m3fn

D = 1024  # model dim
S = 2048  # sequence length
B = 4  # batch
M = 4096  # FFN dim
CH = 256  # q chunk (slot) size
NSLOT = 4  # q slots per core
NDT = D // 128  # 8 d-tiles
N_CORES = 8
NKT = [4, 8, 12, 16]  # k-tiles per slot (static max over the two paired cores)

_PROGRAM = None  # cached compiled program


def _build_program():
    import sys

    if "/opt/trn_rl_repo" not in sys.path:
        sys.path.insert(0, "/opt/trn_rl_repo")
    import concourse.bass as bass
    import concourse.tile as tile
    import concourse.mybir as mybir
    from concourse import bacc
    from concourse.bass import ts

    dt = mybir.dt
    AF = mybir.ActivationFunctionType
    ALU = mybir.AluOpType
    F32, F32R, F16, F8 = dt.float32, dt.float32r, dt.float16, dt.float8e4
    DR = mybir.MatmulPerfMode.DoubleRow

    nc = bacc.Bacc("TRN2", target_bir_lowering=False, debug=False, num_devices=8)

    # ---------------- DRAM I/O (all host-pre-arranged layouts) ----------------
    # fp16 weights: [128, i(8), 1024] with [p, i, c] = W.T[i*128+p, c]
    wvT = nc.dram_tensor("wvT", [128, NDT, D], F16, kind="ExternalInput").ap()
    woT = nc.dram_tensor("woT", [128, NDT, D], F16, kind="ExternalInput").ap()
    # fp8 QK weights in DoubleRow pair layout, host-scaled by 64:
    # [p, i2, s, c] = e4m3(64 * W.T[(2*i2+s)*128+p, c])
    wq8T = nc.dram_tensor("wq8T", [128, 4, 2, D], F8, kind="ExternalInput").ap()
    wk8T = nc.dram_tensor("wk8T", [128, 4, 2, D], F8, kind="ExternalInput").ap()
    # FFN weight: [mb(8), 128, i(8), 512] with [mb, p, i, c] = Wf.T[i*128+p, mb*512+c]
    wfT = nc.dram_tensor("wfT", [M // 512, 128, NDT, 512], F16, kind="ExternalInput").ap()
    # x, own-half tokens in k order: [p, i, t] = x.T[i*128+p, half*1024+t]
    xaT = nc.dram_tensor("xaT", [128, NDT, S // 2], F16, kind="ExternalInput").ap()
    # fp8 x copies in DoubleRow pair layout (for the Q/K projections)
    xa8T = nc.dram_tensor("xa8T", [128, 4, 2, S // 2], F8, kind="ExternalInput").ap()
    xq8T = nc.dram_tensor("xq8T", [128, 4, 2, 4 * CH], F8, kind="ExternalInput").ap()
    bq = nc.dram_tensor("bq", [128, D // 128], F32, kind="ExternalInput").ap()
    bk = nc.dram_tensor("bk", [128, D // 128], F32, kind="ExternalInput").ap()
    bo2 = nc.dram_tensor("bo2", [128, D // 128], F32, kind="ExternalInput").ap()
    bfT = nc.dram_tensor("bfT", [128, M // 128], F32, kind="ExternalInput").ap()
    qpos = nc.dram_tensor("qpos", [1, 4 * CH], F32R, kind="ExternalInput").ap()
    bf_row = nc.dram_tensor("bf_row", [1, M], F32R, kind="ExternalInput").ap()
    iota_kt = nc.dram_tensor("iota_kt", [128, S // 128], F32, kind="ExternalInput").ap()
    # output: [mb(8), 128p, t8(8), 512c] = ff[(t8//4)*512+(t8%4)*128+p, mb*512+c]
    ffT = nc.dram_tensor("ffT", [M // 512, 128, 8, 512], F16, kind="ExternalOutput").ap()

    with tile.TileContext(nc) as tc:
        with (
            tc.tile_pool(name="const", bufs=1) as cpool,
            tc.tile_pool(name="psum", bufs=1, space="PSUM") as pspool,
        ):
            # ---------------- constants (scalar engine issues these) --------
            ones_col_bf = cpool.tile([128, 1], F16, name="ones_col_bf", tag="ones_col_bf")
            nc.vector.memset(ones_col_bf[:], 1.0)
            ones_row_f = cpool.tile([1, 128], F32, name="ones_row_f", tag="ones_row_f")
            nc.vector.memset(ones_row_f[:], 1.0)
            ones_row = cpool.tile([1, 128], F32R, name="ones_row", tag="ones_row")
            nc.vector.tensor_copy(ones_row[:], ones_row_f[:])
            iota_sb = cpool.tile([128, S // 128], F32, name="iota", tag="iota")
            nc.scalar.dma_start(iota_sb[:], iota_kt[:])
            bq_sb = cpool.tile([128, D // 128], F32, name="bq", tag="bq")
            nc.scalar.dma_start(bq_sb[:], bq[:])
            bk_sb = cpool.tile([128, D // 128], F32, name="bk", tag="bk")
            nc.scalar.dma_start(bk_sb[:], bk[:])
            bo2_sb = cpool.tile([128, D // 128], F32, name="bo2", tag="bo2")
            nc.scalar.dma_start(bo2_sb[:], bo2[:])
            bf_sb = cpool.tile([128, M // 128], F32, name="bf", tag="bf")
            nc.scalar.dma_start(bf_sb[:], bfT[:])
            qpos_row = cpool.tile([1, 4 * CH], F32R, name="qpos_row", tag="qpos_row")
            nc.scalar.dma_start(qpos_row[:], qpos[:])
            bfr_sb = cpool.tile([1, M], F32R, name="bfr_sb", tag="bfr_sb")
            nc.scalar.dma_start(bfr_sb[:], bf_row[:])
            qposB = cpool.tile([128, 4 * CH], F32, name="qposB", tag="qposB")
            # V-pass inputs stream from scalar's queue at t0: they follow the
            # (tiny) const DMAs, so the critical wk8/xa8 pieces on gpsimd/sync
            # still see most of the wire, and wv/xa are in well before V needs
            # them (~25us)

            # ------------- long-lived tiles: one pool spanning P1..P4 -------
            with (
                tc.tile_pool(name="main", bufs=1) as mp,
                tc.tile_pool(name="dram", bufs=1, space="DRAM") as dram,
            ):
                # fp8 K^T in DoubleRow pair layout: [p, i2, s, tok],
                # contraction d = (2*i2+s)*128 + p
                kT8 = mp.tile([128, 4, 2, S], F8, name="kT8", tag="kT8")
                vt = mp.tile([128, 16, D], F16, name="vt", tag="vt")
                wo_sb = mp.tile([128, NDT, D], F16, name="wo", tag="wo")
                qT8 = [
                    mp.tile([128, 4, 2, 512], F8, name=f"qT8_{qb}", tag=f"qT8_{qb}")
                    for qb in range(2)
                ]
                attnT = [
                    [mp.tile([128, 512], F16, name=f"at{dt_}_{qb}", tag=f"at{dt_}_{qb}") for qb in range(2)]
                    for dt_ in range(NDT)
                ]
                # DRAM bounce buffers for the pairwise K and V AllGathers.
                # Each projection is gathered in two 1MB halves so the
                # collectives fire earlier and finish well before P2 needs
                # the peer's tokens.
                ka_in = dram.tile([128, 4, 2, 512], F8, name="ka_in", tag="ka_in")
                ka_out = dram.tile([2, 128, 4, 2, 512], F8, name="ka_out", tag="ka_out")
                kb_in = dram.tile([128, 4, 2, 512], F8, name="kb_in", tag="kb_in")
                kb_out = dram.tile([2, 128, 4, 2, 512], F8, name="kb_out", tag="kb_out")
                v_in = dram.tile([128, NDT, D], F16, name="v_in", tag="v_in")
                v_out = dram.tile([2, 128, NDT, D], F16, name="v_out", tag="v_out")

                def pair_gather(in_t, out_t):
                    nc.gpsimd.collective_compute(
                        "AllGather",
                        mybir.AluOpType.bypass,
                        replica_groups=[[0, 1], [2, 3], [4, 5], [6, 7]],
                        ins=[in_t[:].opt()],
                        outs=[out_t[:].opt()],
                    )

                # ---------------- P1 ----------------
                with tc.tile_pool(name="p1a", bufs=1) as p1a:
                    wk8_sb = p1a.tile([128, 4, 2, D], F8, name="wk8", tag="wk8")
                    wq8_sb = p1a.tile([128, 4, 2, D], F8, name="wq8", tag="wq8")
                    wv_sb = p1a.tile([128, NDT, D], F16, name="wv", tag="wv")
                    xa = p1a.tile([128, NDT, S // 2], F16, name="xa", tag="xa")
                    xa8 = p1a.tile([128, 4, 2, S // 2], F8, name="xa8", tag="xa8")
                    xq8 = p1a.tile([128, 4, 2, 4 * CH], F8, name="xq8", tag="xq8")
                    nc.scalar.dma_start(wv_sb[:, 0:4, :], wvT[:, 0:4, :])
                    nc.scalar.dma_start(xa[:, 0:4, :], xaT[:, 0:4, :])

                    # DMA issue is tiered: all in-flight DMAs share wire
                    # bandwidth, so only the critical-path wk/xa stream starts
                    # immediately (per-i granularity: compute starts on the
                    # first 256KB). Later streams are gated on compute
                    # sentinels and issued from the otherwise-idle vector
                    # engine so they cannot steal bandwidth early.
                    for i2 in range(4):
                        nc.gpsimd.dma_start(
                            wk8_sb[:, i2, :, :], wk8T[:, i2, :, :]
                        )
                        nc.sync.dma_start(
                            xa8[:, i2, :, :], xa8T[:, i2, :, :]
                        )


                    # ---- K projection over own half (kT staged at [:, :, 0:1024])
                    for tb in range(2):
                        for og in range(2):
                            ps4 = [
                                pspool.tile([128, 512], F32, name=f"at{j}", tag=f"at{j}", bufs=1)
                                for j in range(4)
                            ]
                            for i2 in range(4):
                                for j in range(4):
                                    nc.tensor.matmul(
                                        ps4[j][:],
                                        wk8_sb[:, i2, :, ts(og * 4 + j, 128)],
                                        xa8[:, i2, :, ts(tb, 512)],
                                        start=(i2 == 0), stop=(i2 == 3),
                                        perf_mode=DR,
                                    )
                            for j in range(4):
                                ot = og * 4 + j
                                nc.scalar.activation(
                                    kT8[:, ot // 2, ot % 2, ts(tb, 512)], ps4[j][:],
                                    AF.Identity, bias=bk_sb[:, ot : ot + 1],
                                    scale=1.0 / 64.0,
                                )
                            if tb == 0 and og == 0:
                                # second halves of the V-pass inputs: issued
                                # only now so they cannot crowd the critical
                                # wk8/xa8 pieces at t0
                                nc.scalar.dma_start(wv_sb[:, 4:8, :], wvT[:, 4:8, :])
                                nc.scalar.dma_start(xa[:, 4:8, :], xaT[:, 4:8, :])
                        # stage this token block and fire its K gather
                        kin = ka_in if tb == 0 else kb_in
                        nc.sync.dma_start(kin[:], kT8[:, :, :, ts(tb, 512)])
                        pair_gather(kin, ka_out if tb == 0 else kb_out)

                    # ---- V projection over own half (token-major, 2 banks/tt)
                    for tb in range(2):
                        for tt in range(4):
                            ps2 = [
                                pspool.tile([128, 512], F32, name=f"mm{ob}", tag="mm", bufs=4)
                                for ob in range(2)
                            ]
                            for i in range(NDT):
                                for ob in range(2):
                                    nc.tensor.matmul(
                                        ps2[ob][:],
                                        xa[:, i, tb * 512 + tt * 128 : tb * 512 + (tt + 1) * 128],
                                        wv_sb[:, i, ts(ob, 512)],
                                        start=(i == 0), stop=(i == NDT - 1),
                                    )
                            for ob in range(2):
                                nc.scalar.activation(
                                    vt[:, tb * 4 + tt, ts(ob, 512)], ps2[ob][:], AF.Identity
                                )
                            if tb == 0 and tt == 0:
                                # tier-2 issue (see tier-1 note)
                                nc.scalar.dma_start(xq8[:], xq8T[:])
                                nc.scalar.dma_start(wq8_sb[:], wq8T[:])
                    # stage own V tokens (vt[:, 0:8] doubles as projection
                    # scratch; readback rewrites all 16 slots in token order)
                    nc.scalar.dma_start(v_in[:], vt[:, 0:NDT, :])
                    pair_gather(v_in, v_out)

                    # qpos broadcast (fills the gather window)
                    for i in range(4 * CH // 512):
                        bc_ps = pspool.tile([128, 512], F32, name="small", tag="at0", bufs=1)
                        nc.tensor.matmul(
                            bc_ps[:], ones_row[:], qpos_row[:, ts(i, 512)],
                            start=True, stop=True,
                        )
                        nc.scalar.activation(qposB[:, ts(i, 512)], bc_ps[:], AF.Identity)

                    # ---- Q projection: qb pair interleaved under one weight
                    for ot in range(NDT):
                        psq = [
                            pspool.tile([128, 512], F32, name=f"mmq{qb}", tag="mm", bufs=4)
                            for qb in range(2)
                        ]
                        for i2 in range(4):
                            for qb in range(2):
                                nc.tensor.matmul(
                                    psq[qb][:],
                                    wq8_sb[:, i2, :, ts(ot, 128)],
                                    xq8[:, i2, :, ts(qb, 512)],
                                    start=(i2 == 0), stop=(i2 == 3),
                                    perf_mode=DR,
                                )
                        for qb in range(2):
                            nc.scalar.activation(
                                qT8[qb][:, ot // 2, ot % 2, :], psq[qb][:],
                                AF.Identity, bias=bq_sb[:, ot : ot + 1],
                                scale=1.0 / 64.0,
                            )
                        if ot == 0:
                            # tier-3: wo is only needed in P3; issuing it this
                            # late keeps it off the K/V gather's wire window
                            for h in range(2):
                                nc.scalar.dma_start(
                                    wo_sb[:, ts(h, 4), :], woT[:, ts(h, 4), :]
                                )

                # gathered K^T / V readback in true token order, ordered by
                # first use in P2 (sync engine; waits ride on the collectives)
                for r in range(2):
                    nc.sync.dma_start(
                        kT8[:, :, :, r * 1024 + 0 : r * 1024 + 512], ka_out[r]
                    )
                    nc.sync.dma_start(
                        kT8[:, :, :, r * 1024 + 512 : r * 1024 + 1024], kb_out[r]
                    )
                for r in range(2):
                    for h in range(2):
                        nc.sync.dma_start(
                            vt[:, r * 8 + h * 4 : r * 8 + (h + 1) * 4, :],
                            v_out[r, :, ts(h, 4), :],
                        )

                # ---------------- P2: attention ----------------
                # Slots are processed as PAIRS sharing a 512-token q block:
                # the k-range both slots need runs at FD=512 (LDWEIGHTS fully
                # hidden), the hi-slot's excess k-tiles at FD=256.
                # pass A computes scores+exp+mask+denom for all slots first
                # (probs stay resident), so the V gather hides behind it.
                with tc.tile_pool(name="p2", bufs=1) as p2:
                    LOHI = [(NKT[0], NKT[1]), (NKT[2], NKT[3])]
                    pt_sh = [
                        [
                            p2.tile([128, 512], F16, name=f"pts{p}_{k}", tag=f"pts{p}_{k}", bufs=1)
                            for k in range(LOHI[p][0])
                        ]
                        for p in range(2)
                    ]
                    pt_ex = [
                        [
                            p2.tile([128, CH], F16, name=f"ptx{p}_{j}", tag=f"ptx{p}_{j}", bufs=1)
                            for j in range(LOHI[p][1] - LOHI[p][0])
                        ]
                        for p in range(2)
                    ]
                    recipB = [
                        p2.tile([128, CH], F32, name=f"recipB{sl}", tag=f"recipB{sl}", bufs=1)
                        for sl in range(NSLOT)
                    ]
                    for p in range(2):
                        lo, hi = LOHI[p]
                        # shared k-range: FD=512 over both slots
                        for k in range(lo):
                            ps = pspool.tile([128, 512], F32, name="mm", tag="mm", bufs=4)
                            for i2 in range(4):
                                nc.tensor.matmul(
                                    ps[:],
                                    kT8[:, i2, :, ts(k, 128)],
                                    qT8[p][:, i2, :, :],
                                    start=(i2 == 0), stop=(i2 == 3),
                                    perf_mode=DR,
                                )
                            nc.scalar.activation(
                                pt_sh[p][k][:], ps[:], AF.Exp, scale=1.0 / 32.0
                            )
                            if k >= lo - 4:
                                # mask applies to the lo slot's columns only
                                msk = p2.tile([128, CH], F16, name="msk", tag="msk", bufs=2)
                                nc.vector.tensor_scalar(
                                    out=msk[:],
                                    in0=qposB[:, ts(2 * p, CH)],
                                    scalar1=iota_sb[:, k : k + 1],
                                    scalar2=None,
                                    op0=ALU.is_ge,
                                )
                                nc.vector.tensor_tensor(
                                    out=pt_sh[p][k][:, 0:CH],
                                    in0=pt_sh[p][k][:, 0:CH], in1=msk[:],
                                    op=ALU.mult,
                                )
                        # excess k-tiles: hi slot only, FD=256, all masked
                        for j, k in enumerate(range(lo, hi)):
                            ps = pspool.tile([128, CH], F32, name="mm", tag="mm", bufs=4)
                            for i2 in range(4):
                                nc.tensor.matmul(
                                    ps[:],
                                    kT8[:, i2, :, ts(k, 128)],
                                    qT8[p][:, i2, :, CH : 2 * CH],
                                    start=(i2 == 0), stop=(i2 == 3),
                                    perf_mode=DR,
                                )
                            praw = p2.tile([128, CH], F16, name="praw", tag="praw", bufs=2)
                            nc.scalar.activation(
                                praw[:], ps[:], AF.Exp, scale=1.0 / 32.0
                            )
                            msk = p2.tile([128, CH], F16, name="msk", tag="msk", bufs=2)
                            nc.vector.tensor_scalar(
                                out=msk[:],
                                in0=qposB[:, ts(2 * p + 1, CH)],
                                scalar1=iota_sb[:, k : k + 1],
                                scalar2=None,
                                op0=ALU.is_ge,
                            )
                            nc.vector.tensor_tensor(
                                out=pt_ex[p][j][:], in0=praw[:], in1=msk[:],
                                op=ALU.mult,
                            )
                        # denominators per slot (FD=256 chains), then recip
                        for h_ in range(2):
                            sl = 2 * p + h_
                            dn_ps = pspool.tile([1, CH], F32, name="small", tag="at0", bufs=1)
                            nmm = LOHI[p][0] if h_ == 0 else hi
                            kk = 0
                            for k in range(lo):
                                nc.tensor.matmul(
                                    dn_ps[:], ones_col_bf[:],
                                    pt_sh[p][k][:, ts(h_, CH)],
                                    start=(kk == 0), stop=(kk == nmm - 1),
                                )
                                kk += 1
                                if h_ == 0 and kk == nmm:
                                    break
                            if h_ == 1:
                                for j in range(hi - lo):
                                    nc.tensor.matmul(
                                        dn_ps[:], ones_col_bf[:], pt_ex[p][j][:],
                                        start=(kk == 0), stop=(kk == nmm - 1),
                                    )
                                    kk += 1
                            dn_r = p2.tile([1, CH], F32R, name="dn_r", tag="dn_r", bufs=2)
                            nc.vector.tensor_copy(dn_r[:], dn_ps[:])
                            rb_ps = pspool.tile([128, CH], F32, name="small2", tag="at1", bufs=1)
                            nc.tensor.matmul(
                                rb_ps[:], ones_row[:], dn_r[:], start=True, stop=True
                            )
                            nc.vector.reciprocal(recipB[sl][:], rb_ps[:])

                    # pass B: attn^T = (P @ V)^T scaled by 1/denom
                    for p in range(2):
                        lo, hi = LOHI[p]
                        for half in range(2):
                            for d4 in range(4):
                                d_ = half * 4 + d4
                                aps = pspool.tile([128, 512], F32, name=f"at{d4}", tag=f"at{d4}", bufs=1)
                                for k in range(lo):
                                    nc.tensor.matmul(
                                        aps[:],
                                        vt[:, k, ts(d_, 128)],
                                        pt_sh[p][k][:],
                                        start=(k == 0), stop=False,
                                        skip_group_check=True,
                                    )
                                for j, k in enumerate(range(lo, hi)):
                                    nc.tensor.matmul(
                                        aps[:, CH : 2 * CH],
                                        vt[:, k, ts(d_, 128)],
                                        pt_ex[p][j][:],
                                        start=False, stop=(k == hi - 1),
                                        skip_group_check=True,
                                    )
                                if hi == lo:
                                    pass
                                for h_ in range(2):
                                    nc.vector.tensor_tensor(
                                        out=attnT[d_][p][:, ts(h_, CH)],
                                        in0=aps[:, ts(h_, CH)],
                                        in1=recipB[2 * p + h_][:],
                                        op=ALU.mult,
                                    )

                # ---------------- P3 + P4 ----------------
                with tc.tile_pool(name="p34", bufs=1) as p34:
                    outT = [
                        [p34.tile([128, 512], F16, name=f"oT{dt_}_{qb}", tag=f"oT{dt_}_{qb}") for qb in range(2)]
                        for dt_ in range(NDT)
                    ]
                    bfbc = p34.tile([128, 8, 512], F16, name="bfbc", tag="bfbc")
                    for mb in range(M // 512):
                        bc_ps = pspool.tile([128, 512], F32, name="small", tag="at0", bufs=1)
                        nc.tensor.matmul(
                            bc_ps[:], ones_row[:], bfr_sb[:, ts(mb, 512)],
                            start=True, stop=True,
                        )
                        nc.scalar.activation(bfbc[:, mb, :], bc_ps[:], AF.Identity)
                    # P3: qb pair interleaved under one Wo weight block
                    for ot in range(NDT):
                        pso = [
                            pspool.tile([128, 512], F32, name=f"mmo{qb}", tag="mm", bufs=4)
                            for qb in range(2)
                        ]
                        for i in range(NDT):
                            for qb in range(2):
                                nc.tensor.matmul(
                                    pso[qb][:],
                                    wo_sb[:, i, ts(ot, 128)],
                                    attnT[i][qb][:],
                                    start=(i == 0), stop=(i == NDT - 1),
                                )
                        for qb in range(2):
                            nc.scalar.activation(
                                outT[ot][qb][:], pso[qb][:], AF.Identity,
                                bias=bo2_sb[:, ot : ot + 1],
                            )

                    # P4: FFN + GELU. Stationary = outT token-blocks, moving
                    # = the full 512-wide wf block: LDWEIGHTS hides under the
                    # previous matmul (K-pass pattern, 8-bank rotation). The
                    # free-dim bias bf is pre-filled into PSUM by the vector
                    # engine; matmuls accumulate on top (start=False).
                    for mb in range(M // 512):
                        wfb = p34.tile([128, NDT, 512], F16, name="wfb", tag="wfb", bufs=2)
                        nc.gpsimd.dma_start(wfb[:], wfT[mb])
                        st = p34.tile([128, 8, 512], F16, name="ffstage", tag="ffstage", bufs=2)
                        ps8 = [
                            pspool.tile(
                                [128, 512], F32, name=f"ps8_{t8}",
                                tag=(f"at{t8}" if t8 < 4 else "mm"),
                                bufs=(1 if t8 < 4 else 4),
                            )
                            for t8 in range(8)
                        ]
                        for t8 in range(8):
                            qb, tb2 = divmod(t8, 4)
                            nc.vector.tensor_copy(ps8[t8][:], bfbc[:, mb, :])
                            for i in range(NDT):
                                nc.tensor.matmul(
                                    ps8[t8][:],
                                    outT[i][qb][:, ts(tb2, 128)],
                                    wfb[:, i, :],
                                    start=False, stop=(i == NDT - 1),
                                    skip_group_check=True,
                                )
                            nc.scalar.activation(st[:, t8, :], ps8[t8][:], AF.Gelu)
                            if t8 == 3:
                                nc.sync.dma_start(ffT[mb, :, 0:4, :], st[:, 0:4, :])
                        nc.sync.dma_start(ffT[mb, :, 4:8, :], st[:, 4:8, :])

    nc.compile()
    return nc


def _get_program():
    global _PROGRAM
    if _PROGRAM is None:
        _PROGRAM = _build_program()
    return _PROGRAM


def _owned_chunks(core):
    """The four 256-token chunk indices this core owns, in slot order."""
    if core % 2 == 0:
        return (0, 3, 4, 7)
    return (1, 2, 5, 6)


def _blocked(a):
    """[1024, W] -> [128, 8, W] with [p, i, c] = a[i*128+p, c]."""
    W = a.shape[1]
    return np.ascontiguousarray(a.reshape(8, 128, W).transpose(1, 0, 2))


def _pair8(a, scale=1.0):
    """[1024, W] -> [128, 4, 2, W] e4m3 with [p, i2, s, c] = scale*a[(2i2+s)*128+p, c]."""
    W = a.shape[1]
    t = (np.asarray(a, np.float32) * scale).reshape(4, 2, 128, W).transpose(2, 0, 1, 3)
    return np.ascontiguousarray(t).astype(_E4M3)


def _make_in_maps(x, Wq, bq, Wk, bk, Wv, bv, Wo, bo, Wf, bf):
    f32, f16 = np.float32, np.float16
    wq8T = _pair8(np.asarray(Wq.T, np.float32), 64.0)
    wk8T = _pair8(np.asarray(Wk.T, np.float32), 64.0)
    wvT = _blocked(np.asarray(Wv.T, dtype=f16))
    woT = _blocked(np.asarray(Wo.T, dtype=f16))
    # wfT[mb, p, i, c] = Wf.T[i*128+p, mb*512+c]
    wfT = np.ascontiguousarray(
        np.asarray(Wf.T, dtype=f16).reshape(8, 128, 8, 512).transpose(2, 1, 0, 3)
    )
    bo2 = (Wo.astype(np.float64) @ bv.astype(np.float64) + bo.astype(np.float64))
    bo2 = np.ascontiguousarray(bo2.astype(f32).reshape(D // 128, 128).T)
    bfT = np.ascontiguousarray(bf.reshape(M // 128, 128).T, dtype=f32)
    iota = (
        np.arange(128, dtype=f32)[:, None]
        + 128.0 * np.arange(S // 128, dtype=f32)[None, :]
    )
    shared = {
        "wq8T": wq8T, "wk8T": wk8T, "wvT": wvT, "woT": woT, "wfT": wfT,
        "bq": np.ascontiguousarray(bq.reshape(D // 128, 128).T, dtype=f32),
        "bk": np.ascontiguousarray(bk.reshape(D // 128, 128).T, dtype=f32),
        "bo2": bo2,
        "bfT": bfT,
        "bf_row": np.ascontiguousarray(bf[None, :].astype(f32)),
        "iota_kt": np.ascontiguousarray(iota),
    }
    in_maps = []
    for core in range(N_CORES):
        b = core // 2
        chunks = _owned_chunks(core)
        xTb = np.asarray(x[b].T, dtype=f16)  # [D, S]
        half = core % 2  # rank within the pair: rank0 owns tokens 0:S/2
        xown = xTb[:, half * (S // 2) : (half + 1) * (S // 2)]
        xaT = _blocked(xown)
        xa8T = _pair8(xown.astype(np.float32))
        xq8T = _pair8(
            np.concatenate(
                [xTb[:, c * CH : (c + 1) * CH] for c in chunks], axis=1
            ).astype(np.float32)
        )
        qp = np.concatenate(
            [np.arange(c * CH, (c + 1) * CH) for c in chunks]
        ).astype(f32)[None, :]
        in_maps.append(
            {**shared, "xaT": xaT, "xa8T": xa8T, "xq8T": xq8T,
             "qpos": np.ascontiguousarray(qp)}
        )
    return in_maps


def _run(inputs, trace=False, trace_cores=None, tmpdir=None):
    import sys

    if "/opt/trn_rl_repo" not in sys.path:
        sys.path.insert(0, "/opt/trn_rl_repo")
    from concourse.bass_utils import run_bass_kernel_spmd

    nc = _get_program()
    in_maps = _make_in_maps(**inputs)
    res = run_bass_kernel_spmd(
        nc, in_maps, list(range(N_CORES)), trace=trace, trace_cores=trace_cores,
        tmpdir=tmpdir,
    )
    out = np.empty((B, S, M), dtype=np.float32)
    for core in range(N_CORES):
        b = core // 2
        chunks = _owned_chunks(core)
        # ffT[mb, p, t8, c] = ff[(t8//4)*512 + (t8%4)*128 + p, mb*512 + c]
        raw = res.results[core]["ffT"].reshape(8, 128, 2, 4, 512)
        ff = np.ascontiguousarray(
            raw.transpose(2, 3, 1, 0, 4)
        ).reshape(4 * CH, M)
        for sl, c in enumerate(chunks):
            qb, qo = divmod(sl, 2)
            out[b, c * CH : (c + 1) * CH] = (
                ff[qb * 512 + qo * CH : qb * 512 + (qo + 1) * CH].astype(np.float32)
            )
    return out, res


def kernel(**inputs):
    out, _ = _run(inputs)
    return out
